# revision 36
# baseline (speedup 1.0000x reference)
"""Multi-head attention (B=2, S=2048, D=1024, H=16) on 8 Trainium2 cores.

Sharding: data-parallel over batch (2 groups of 4 cores) x tensor-parallel
over heads (4 heads per core, organized as 2 pairs). Each core:
  - projects its 4 heads' q/k/v (bf16 matmuls; K bias dropped -- it only
    shifts each query's scores by a constant, which softmax cancels),
  - computes scores^T = K_h Q_h^T / 8 per head with the two heads of a
    pair emitted as adjacent matmuls on PE row groups 0-1 / 2-3 (they run
    concurrently -- 2x effective rate on the 64-deep contraction),
  - exp on ScalarE in 3-bank PSUM chunks (amortizes the ~300-cycle
    per-instruction overhead),
  - attended^T = [V_h | 1]^T P^T -- the appended ones column yields the
    softmax denominators for free in PSUM row 64,
  - normalizes via one batched reciprocal per q-chunk ([4,512] covering
    all 4 heads -- DVE reciprocal is ~8 cyc/elem along the free dim, so
    batching heads 4x-es it) + rank-1 PE broadcast per pair,
  - row-parallel output projection producing a partial out^T [D, S] in
    bf16.
Host sums the 4 partials per batch, transposes, and adds the constant
bias vector bo + bv @ Wo^T (the V bias commutes through softmax because
the attention weights sum to 1).
"""

import sys

if '/opt/trn_rl_repo' not in sys.path:
    sys.path.insert(0, '/opt/trn_rl_repo')

import numpy as np

import concourse.bass as bass
import concourse.mybir as mybir
import concourse.tile as tile

# ---------------------------------------------------------------------------
# Workaround: the walrus build in this container accepts only one sync-wait
# per instruction. Hoist excess waits onto single-wait NoOp carriers, and
# emit the Tile tail-drain waits as individual SP instructions.
# ---------------------------------------------------------------------------
from concourse.vector_clock import ScopedClock

_MAXW = 1
_carrier_counter = [0]


def _split_excess_waits(tc, ordered):
    for insts in ordered.values():
        out = []
        for inst in insts:
            si = inst.sync_info
            waits = list(si.on_wait) if si is not None and si.on_wait else []
            if len(waits) > _MAXW:
                for w in waits[_MAXW:]:
                    _carrier_counter[0] += 1
                    out.append(mybir.InstNoOp(
                        name=f"I-waitcarrier-{_carrier_counter[0]}",
                        engine=inst.engine,
                        sync_info=mybir.SyncInfo(on_wait=[w], on_update=[]),
                        bass_nofuse=True,
                    ))
                inst.sync_info = mybir.SyncInfo(
                    on_wait=waits[:_MAXW],
                    on_update=list(si.on_update) if si.on_update else [],
                )
            out.append(inst)
        if len(out) != len(insts):
            insts[:] = out


class _SplitTileClockWait:
    def __init__(self, tc, ordered):
        self._w = _OrigTileClockWait(tc, ordered)
        self._tc = tc
        self._ordered = ordered

    def assign_waits(self, bb_name):
        r = self._w.assign_waits(bb_name)
        _split_excess_waits(self._tc, self._ordered)
        return r

    def __getattr__(self, name):
        return getattr(self._w, name)


def _patched_drain_and_barrier(self, tick_clock, wait_clock):
    nc = self.nc
    probe = mybir.InstNoOp(
        name=nc.get_next_instruction_name(), engine=mybir.EngineType.SP
    )
    wait_clock.add_sem_waits(probe, ScopedClock({None: tick_clock.global_clock}))
    waits = list(probe.sync_info.on_wait) if probe.sync_info is not None else []
    assert self.sems is not None
    allocated = list(self.sems.allocated().values())
    id2handle = {h.num: h for h in allocated}
    for w in waits:
        nc.sync.wait_ge(id2handle[w.id], w.wait_value)
    nc.sync.drain()
    nc.all_engine_barrier()
    popped = nc._tile_sem_poison_stack.pop()
    assert popped is self._sem_poison
    nc.clear_and_free_semaphores(allocated)
    nc.all_engine_barrier()


_OrigTileClockWait = None


def _apply_tilefix():
    global _OrigTileClockWait
    if _OrigTileClockWait is None:
        _OrigTileClockWait = tile.TileClockWait
        tile.TileClockWait = _SplitTileClockWait
        tile.TileContext._drain_and_barrier = _patched_drain_and_barrier


_apply_tilefix()

# ---------------------------------------------------------------------------
# Problem constants
# ---------------------------------------------------------------------------
F32 = mybir.dt.float32
F32R = mybir.dt.float32r
BF16 = mybir.dt.bfloat16
FP8 = mybir.dt.float8e4
EXP = mybir.ActivationFunctionType.Exp

# Q/K projections in fp8e4m3 with DoubleRow (2 contraction tiles per
# matmul). Host ships Wq/Wk scaled by 16 (their native ±1/32 range would
# land in fp8 subnormals); the 16*16 factor on the scores is folded into
# the exp() scale. V stays bf16: value-quantization error passes straight
# through to the output, score error is softened by softmax.
QK_FP8 = True
QK_SCALE = 16.0

B, S, D, H = 2, 2048, 1024, 16
DH = D // H                    # 64
NCORES = 8
GROUPS = 4                     # head groups (cores per batch)
HPG = H // GROUPS              # 4 heads per core
PAIRS = HPG // 2               # 2 head pairs per core
MW = HPG * DH                  # 256: per-core projection width
KC = D // 128                  # 8 contraction chunks for the projections
MC = MW // 128                 # 2 partition-chunks of the head dim (== pair)
QBLK = 512


def build_program(seq=S, loop_iters=None, phases=('proj', 'attn', 'out'),
                  xbufs=3, scb=3, sbufs=2, accbufs=2, ptbufs=3,
                  dup_every=2):
    """Emit the per-core Bass program. seq can be shrunk for simulation.

    scb: scores PSUM tile size in banks (matmuls per exp instruction).
    dup_every: if k>0, emit a throwaway duplicate of every k-th scores
    matmul (PE filler to keep the HAM clock-gate warm when ScalarE gates).
    """
    assert seq % QBLK == 0
    SC = seq // QBLK            # s-chunks (projection streaming)
    QC = seq // QBLK            # q-chunks (attention)
    KT = seq // 128             # key-row tiles
    ET = D // 128               # output-feature tiles

    nc = bass.Bass("TRN2", target_bir_lowering=False, debug=False,
                   num_devices=NCORES)
    dt_qk = FP8 if QK_FP8 else BF16
    xqT = nc.dram_tensor("xqT", [D, seq], dt_qk, kind="ExternalInput").ap()
    xkT = nc.dram_tensor("xkT", [D, seq], dt_qk, kind="ExternalInput").ap()
    xvT = nc.dram_tensor("xvT", [D, seq], BF16, kind="ExternalInput").ap()
    wqT = nc.dram_tensor("wqT", [D, MW], dt_qk, kind="ExternalInput").ap()
    wkT = nc.dram_tensor("wkT", [D, MW], dt_qk, kind="ExternalInput").ap()
    wvT = nc.dram_tensor("wvT", [D, MW], BF16, kind="ExternalInput").ap()
    woT = nc.dram_tensor("woT", [MW, D], BF16, kind="ExternalInput").ap()
    bq = nc.dram_tensor("bq", [MW], F32, kind="ExternalInput").ap()
    ebc = nc.dram_tensor("ebc", [128, 2 * 128], BF16,
                         kind="ExternalInput").ap()
    outT = nc.dram_tensor("outT", [D, seq], BF16, kind="ExternalOutput").ap()

    with tile.TileContext(nc) as tc:
        with (
            tc.tile_pool(name="w", bufs=1) as wpool,
            tc.tile_pool(name="x", bufs=xbufs) as xpool,
            tc.tile_pool(name="qkv", bufs=1) as qkvp,
            tc.tile_pool(name="pt", bufs=ptbufs) as ptp,
            tc.tile_pool(name="attn", bufs=2) as attnp,
            tc.tile_pool(name="io", bufs=2) as iop,
            tc.tile_pool(name="r", bufs=2) as rp,
            tc.tile_pool(name="ps", bufs=1, space="PSUM") as psp,
        ):
            def body():
                # --- weights + biases resident ---
                wq_sb = wpool.tile([128, KC, MW], dt_qk, tag="wq")
                wk_sb = wpool.tile([128, KC, MW], dt_qk, tag="wk")
                wv_sb = wpool.tile([128, KC, MW], BF16, tag="wv")
                wo_sb = wpool.tile([128, MC, D], BF16, tag="wo")
                bq_sb = wpool.tile([128, MC], F32, tag="bq")

                def load_w(kind):
                    if kind == "k":
                        nc.sync.dma_start(
                            out=wk_sb[:],
                            in_=wkT.rearrange("(kc p) m -> p kc m", p=128))
                    elif kind == "q":
                        nc.sync.dma_start(
                            out=wq_sb[:],
                            in_=wqT.rearrange("(kc p) m -> p kc m", p=128))
                        nc.sync.dma_start(
                            out=bq_sb[:],
                            in_=bq.rearrange("(mc p) -> p mc", p=128))
                    elif kind == "v":
                        nc.sync.dma_start(
                            out=wv_sb[:],
                            in_=wvT.rearrange("(kc p) m -> p kc m", p=128))
                        nc.sync.dma_start(
                            out=wo_sb[:],
                            in_=woT.rearrange("(mc p) e -> p mc e", p=128))

                qT_sb = qkvp.tile([128, MC, seq], BF16, tag="qT")
                kT_sb = qkvp.tile([128, MC, seq], BF16, tag="kT")
                v_sb = qkvp.tile([128, KT, HPG, DH + 1], BF16, tag="v")
                ones_src = wpool.tile([128, KT * HPG], F32, tag="ones")
                nc.vector.memset(ones_src[:], 1.0)
                nc.vector.tensor_copy(
                    v_sb[:, :, :, DH],
                    ones_src[:].rearrange("p (kt h) -> p kt h", h=HPG))
                # Pair-broadcast selector (host-built): Ep[p][i, c] = 1 iff
                # row i (= partition 32*h of the reciprocal tile r4) belongs
                # to the head owning partition c of pair p. rb = Ep^T @ r4
                # broadcasts each head's 1/den row across its 64 partitions.
                # Denominators sit on 32-aligned partitions because DVE
                # accesses must start on a quadrant boundary.
                e_sb = wpool.tile([128, 2, 128], BF16, tag="e")
                nc.sync.dma_start(
                    out=e_sb[:], in_=ebc.rearrange("r (mc c) -> r mc c", c=128))

                # --- projections ---
                if 'proj' not in phases:
                    return

                loaded_w = set()

                def emit_proj(kind, xdram, w_sb, scs=None):
                    if kind not in loaded_w:
                        loaded_w.add(kind)
                        load_w(kind)
                    for sc in (range(SC) if scs is None else scs):
                        x_sb = xpool.tile([128, KC, QBLK],
                                          BF16 if kind == "v" else dt_qk,
                                          tag="x" if kind == "v" else "x8")
                        nc.sync.dma_start(
                            out=x_sb[:],
                            in_=xdram.rearrange("(kc p) s -> p kc s", p=128)
                            [:, :, sc * QBLK:(sc + 1) * QBLK])
                        if 'nomm' in phases:
                            continue
                        if kind != "v":
                            dest = qT_sb if kind == "q" else kT_sb
                            w_sb = wq_sb if kind == "q" else wk_sb
                            for mc in range(MC):
                                ps = psp.tile([128, QBLK], F32, tag="acc",
                                              bufs=accbufs)
                                ms = slice(mc * 128, (mc + 1) * 128)
                                if QK_FP8:
                                    for kc2 in range(KC // 2):
                                        nc.tensor.matmul(
                                            ps[:],
                                            w_sb[:, 2 * kc2:2 * kc2 + 2, ms],
                                            x_sb[:, 2 * kc2:2 * kc2 + 2, :],
                                            start=(kc2 == 0),
                                            stop=(kc2 == KC // 2 - 1),
                                            perf_mode=(
                                                mybir.MatmulPerfMode.DoubleRow))
                                else:
                                    for kc in range(KC):
                                        nc.tensor.matmul(
                                            ps[:], w_sb[:, kc, ms],
                                            x_sb[:, kc, :],
                                            start=(kc == 0),
                                            stop=(kc == KC - 1))
                                dst = dest[:, mc, sc * QBLK:(sc + 1) * QBLK]
                                if kind == "q":
                                    nc.vector.tensor_scalar_add(
                                        dst, ps[:], bq_sb[:, mc:mc + 1])
                                else:
                                    nc.vector.tensor_copy(dst, ps[:])
                        else:
                            for st in range(QBLK // 128):
                                ps = psp.tile([128, QBLK], F32, tag="acc",
                                              bufs=accbufs)
                                for kc in range(KC):
                                    nc.tensor.matmul(
                                        ps[:, 0:MW],
                                        x_sb[:, kc, st * 128:(st + 1) * 128],
                                        wv_sb[:, kc, :],
                                        start=(kc == 0), stop=(kc == KC - 1))
                                kt = sc * (QBLK // 128) + st
                                nc.vector.tensor_copy(
                                    v_sb[:, kt, :, 0:DH],
                                    ps[:, 0:MW].rearrange(
                                        "p (h d) -> p h d", h=HPG))

                if 'attn' not in phases:
                    emit_proj("k", xkT, wk_sb)
                    emit_proj("q", xqT, wq_sb)
                    emit_proj("v", xvT, wv_sb)
                    return
                emit_proj("k", xkT, wk_sb, scs=[0])

                # --- attention ---
                def emit_scores_pair(pair, qc, pts, j_lo=0, j_hi=None):
                    """Scores + exp for both heads of a pair, interleaved so
                    the 64-contraction matmuls overlap on PE row groups.
                    j indexes (kt, head) slots; a sub-range lets the qc=0
                    scores chase the K projection chunk by chunk."""
                    if j_lo == 0:
                        pts[pair] = ptp.tile([128, 2 * KT, QBLK], BF16,
                                             tag="pt", name=f"pt{pair}_{qc}")
                    pt = pts[pair]
                    qs = slice(qc * QBLK, (qc + 1) * QBLK)
                    n = 2 * KT if j_hi is None else j_hi
                    j = j_lo
                    while j < n:
                        w = min(scb, n - j)
                        ps_s = psp.tile([128, scb, QBLK], F32, tag="s",
                                        bufs=sbufs)
                        for i in range(w):
                            kt, half = divmod(j + i, 2)
                            lo = half * 64
                            if dup_every and ((j + i) % dup_every == 0):
                                nc.tensor.matmul(
                                    ps_s[:, i, :],
                                    kT_sb[lo:lo + 64, pair,
                                          kt * 128:(kt + 1) * 128],
                                    qT_sb[lo:lo + 64, pair, qs],
                                    start=True, stop=True)
                            nc.tensor.matmul(
                                ps_s[:, i, :],
                                kT_sb[lo:lo + 64, pair,
                                      kt * 128:(kt + 1) * 128],
                                qT_sb[lo:lo + 64, pair, qs],
                                start=True, stop=True)
                        nc.scalar.activation(
                            pt[:, j:j + w, :], ps_s[:, 0:w, :],
                            EXP, scale=1.0 / np.sqrt(DH) / (
                                QK_SCALE * QK_SCALE if QK_FP8 else 1.0))
                        j += w

                def emit_pv_pair(pair, qc, pt, pv_sb, den4):
                    """P@V for both heads of a pair; values land in pv_sb
                    (head A rows 0-63, head B rows 64-127), denominators on
                    32-aligned den4 partitions 32*(2*pair) / 32*(2*pair+1)."""
                    for half in range(2):
                        h = 2 * pair + half
                        ps_pv = psp.tile([128, QBLK], F32, tag="acc",
                                         bufs=accbufs)
                        for kt in range(KT):
                            nc.tensor.matmul(
                                ps_pv[0:DH + 1, :], v_sb[:, kt, h, :],
                                pt[:, 2 * kt + half, :],
                                start=(kt == 0), stop=(kt == KT - 1))
                        nc.vector.tensor_copy(
                            pv_sb[half * 64:(half + 1) * 64, :],
                            ps_pv[0:DH, :])
                        nc.vector.tensor_copy(
                            den4[32 * h:32 * h + 1, :], ps_pv[DH:DH + 1, :])

                def emit_recip(den4, r4s, qc):
                    # One reciprocal covers all 4 heads' denominators (DVE
                    # reciprocal cost is free-dim-serial, partition-parallel;
                    # rows other than 32h hold the memset 1.0 filler). The
                    # broadcast+multiply consume it one iteration later, and
                    # it is emitted in 4 chunks: DVE reciprocal runs ~7
                    # cyc/elem and a monolithic one head-of-line-blocks the
                    # DVE queue, starving PE of freed PSUM accumulators.
                    r4 = rp.tile([128, QBLK], BF16, tag="r4", bufs=2,
                                 name=f"r4_{qc}")
                    r4s[qc] = r4
                    with nc.allow_low_precision(reason="bf16 denom bcast"):
                        for c in range(4):
                            cs = slice(c * QBLK // 4, (c + 1) * QBLK // 4)
                            nc.vector.reciprocal(r4[:, cs], den4[:, cs])

                def emit_bcmul(qc, attn_sb, pv_sbs, r4):
                    for pair in range(PAIRS):
                        rb_ps = psp.tile([128, QBLK], F32, tag="acc",
                                         bufs=accbufs)
                        nc.tensor.matmul(rb_ps[:], e_sb[:, pair, :], r4[:],
                                         start=True, stop=True)
                        nc.vector.tensor_mul(
                            attn_sb[:, pair, :], pv_sbs[pair][:], rb_ps[:])

                def emit_outproj(qc, attn_sb, ets=None, scalar_copy=False):
                    for et in (range(ET) if ets is None else ets):
                        ps_o = psp.tile([128, QBLK], F32, tag="acc",
                                        bufs=accbufs)
                        for mc in range(MC):
                            nc.tensor.matmul(
                                ps_o[:],
                                wo_sb[:, mc, et * 128:(et + 1) * 128],
                                attn_sb[:, mc, :],
                                start=(mc == 0), stop=(mc == MC - 1))
                        ot = iop.tile([128, QBLK], BF16, tag="ot")
                        if scalar_copy:
                            # tail iterations: exp stream is finished, so
                            # the idle ScalarE drains PSUM instead of DVE
                            nc.scalar.copy(ot[:], ps_o[:])
                        else:
                            nc.vector.tensor_copy(ot[:], ps_o[:])
                        nc.sync.dma_start(
                            out=outT.rearrange("(et p) q -> p et q", p=128)
                            [:, et, qc * QBLK:(qc + 1) * QBLK],
                            in_=ot[:])

                # qc=0: scores chase the K projection chunk by chunk so
                # ScalarE starts exping within the first few us; the V
                # projection and remaining Q chunks keep PE fed while
                # ScalarE works through the qc=0 exps.
                emit_proj("q", xqT, wq_sb, scs=[0])
                ptss = {0: {}}
                KJ = 2 * (QBLK // 128)   # (kt, head) slots per K s-chunk
                for sc in range(SC):
                    if sc > 0:
                        emit_proj("k", xkT, wk_sb, scs=[sc])
                    for pair in range(PAIRS):
                        emit_scores_pair(pair, 0, ptss[0],
                                         j_lo=sc * KJ, j_hi=(sc + 1) * KJ)
                emit_proj("v", xvT, wv_sb)
                if SC > 1:
                    emit_proj("q", xqT, wq_sb, scs=[1])

                attns = {}
                pvs = {}
                r4s = {}
                for qc in range(1, QC + 1):
                    cur = qc <= QC - 1
                    if cur:
                        ptss[qc] = {}
                    prev = qc - 1
                    pvs[prev] = [
                        rp.tile([128, QBLK], F32R, tag="pv", bufs=4,
                                name=f"pv{prev}_{p}") for p in range(PAIRS)]
                    den4 = rp.tile([128, QBLK], BF16, tag="den4",
                                   bufs=2, name=f"den{prev}")
                    nc.vector.memset(den4[:], 1.0)
                    for pair in range(PAIRS):
                        if cur:
                            emit_scores_pair(pair, qc, ptss[qc])
                        emit_pv_pair(pair, prev, ptss[prev].pop(pair),
                                     pvs[prev][pair], den4)
                        if pair == 0:
                            if qc >= 2:
                                attns[qc - 2] = attnp.tile(
                                    [128, MC, QBLK], BF16, tag="attn",
                                    name=f"attn{qc - 2}")
                                emit_bcmul(qc - 2, attns[qc - 2],
                                           pvs.pop(qc - 2), r4s.pop(qc - 2))
                            if qc + 1 <= SC - 1:
                                emit_proj("q", xqT, wq_sb, scs=[qc + 1])
                            if qc >= 2 and 'out' in phases:
                                emit_outproj(qc - 2, attns[qc - 2],
                                             ets=range(ET // 2))
                        else:
                            if qc >= 2 and 'out' in phases:
                                emit_outproj(qc - 2, attns.pop(qc - 2),
                                             ets=range(ET // 2, ET),
                                             scalar_copy=(qc == QC))
                            emit_recip(den4, r4s, prev)
                # tail: last q-chunk's normalize + output projection
                attns[QC - 1] = attnp.tile([128, MC, QBLK], BF16,
                                           tag="attn", name=f"attn{QC - 1}")
                emit_bcmul(QC - 1, attns[QC - 1], pvs.pop(QC - 1),
                           r4s.pop(QC - 1))
                if 'out' in phases:
                    emit_outproj(QC - 1, attns.pop(QC - 1), scalar_copy=True)

            if loop_iters is not None:
                with tc.For_i(0, loop_iters, 1,
                              staggered_reset=True,
                              hint_engines=(mybir.EngineType.PE,
                                            mybir.EngineType.DVE,
                                            mybir.EngineType.Activation,
                                            mybir.EngineType.SP)):
                    body()
            else:
                body()

    return nc


# ---------------------------------------------------------------------------
# Host-side sharding / unsharding
# ---------------------------------------------------------------------------

def shard_inputs(query, keys, values, Wq, bq, Wk, bk, Wv, bv, Wo, bo):
    import ml_dtypes
    bf16 = ml_dtypes.bfloat16
    qk_dt = ml_dtypes.float8_e4m3fn if QK_FP8 else bf16
    qk_ws = QK_SCALE if QK_FP8 else 1.0
    ebc = np.zeros((128, PAIRS, 128), np.float32)
    for pair in range(PAIRS):
        ebc[32 * (2 * pair), pair, 0:64] = 1.0
        ebc[32 * (2 * pair) + 32, pair, 64:128] = 1.0
    ebc = ebc.reshape(128, PAIRS * 128)
    in_maps = []
    for c in range(NCORES):
        b, g = divmod(c, GROUPS)
        cols = slice(g * MW, (g + 1) * MW)
        in_maps.append({
            "xqT": np.ascontiguousarray(np.asarray(query)[b].T).astype(qk_dt),
            "xkT": np.ascontiguousarray(np.asarray(keys)[b].T).astype(qk_dt),
            "xvT": np.ascontiguousarray(np.asarray(values)[b].T).astype(bf16),
            "wqT": np.ascontiguousarray(
                np.asarray(Wq)[cols].T * qk_ws).astype(qk_dt),
            "wkT": np.ascontiguousarray(
                np.asarray(Wk)[cols].T * qk_ws).astype(qk_dt),
            "wvT": np.ascontiguousarray(np.asarray(Wv)[cols].T).astype(bf16),
            "woT": np.ascontiguousarray(
                np.asarray(Wo)[:, cols].T).astype(bf16),
            "bq": np.ascontiguousarray(np.asarray(bq)[cols] * qk_ws),
            "ebc": ebc.astype(bf16),
        })
    return in_maps


def unshard(results, Wo, bv, bo):
    const = np.asarray(bo) + np.asarray(bv) @ np.asarray(Wo).T
    out = np.zeros((B, S, D), np.float32)
    for c in range(NCORES):
        b = c // GROUPS
        out[b] += results[c]["outT"].astype(np.float32).T
    out += const.astype(np.float32)
    return out


# ---------------------------------------------------------------------------
# Cached PJRT runner (compile once per process)
# ---------------------------------------------------------------------------

class Runner:
    def __init__(self, nc):
        import jax
        from concourse import bass2jax
        from jax.experimental.shard_map import shard_map
        from jax.sharding import Mesh, PartitionSpec

        bass2jax.install_neuronx_cc_hook()
        self._jax = jax
        partition_name = (nc.partition_id_tensor.name
                          if nc.partition_id_tensor else None)
        in_names, out_names, out_avals = [], [], []
        self._zero_templates = []
        for alloc in nc.m.functions[0].allocations:
            if not isinstance(alloc, mybir.MemoryLocationSet):
                continue
            name = alloc.memorylocations[0].name
            if alloc.kind == "ExternalInput":
                if name == partition_name:
                    continue
                in_names.append(name)
            elif alloc.kind == "ExternalOutput":
                out_names.append(name)
                shape = tuple(alloc.tensor_shape)
                dtype = mybir.dt.np(alloc.dtype)
                out_avals.append(jax.core.ShapedArray(shape, dtype))
                self._zero_templates.append((shape, dtype))
        self._in_names = list(in_names)
        self._out_names = list(out_names)
        self._out_avals = out_avals
        n_params = len(in_names)
        n_outs = len(out_names)
        all_in_names = in_names + out_names
        if partition_name is not None:
            all_in_names = all_in_names + [partition_name]

        def _body(*args):
            operands = list(args)
            if partition_name is not None:
                operands.append(bass2jax.partition_id_tensor())
            outs = bass2jax._bass_exec_p.bind(
                *operands,
                out_avals=tuple(out_avals),
                in_names=tuple(all_in_names),
                out_names=tuple(out_names),
                lowering_input_output_aliases=(),
                sim_require_finite=True,
                sim_require_nnan=True,
                nc=nc,
            )
            return tuple(outs)

        devices = jax.devices()[:NCORES]
        mesh = Mesh(np.asarray(devices), ("core",))
        donate = tuple(range(n_params, n_params + n_outs))
        self._fn = jax.jit(
            shard_map(_body, mesh=mesh,
                      in_specs=(PartitionSpec("core"),) * (n_params + n_outs),
                      out_specs=(PartitionSpec("core"),) * n_outs,
                      check_rep=False),
            donate_argnums=donate, keep_unused=True)

    def run(self, in_maps):
        concat_in = [
            np.concatenate([np.asarray(m[name]) for m in in_maps], axis=0)
            for name in self._in_names
        ]
        concat_zeros = [
            np.zeros((NCORES * shape[0], *shape[1:]), dtype)
            for shape, dtype in self._zero_templates
        ]
        out_arrs = self._fn(*concat_in, *concat_zeros)
        return [
            {
                name: np.asarray(out_arrs[i]).reshape(
                    NCORES, *self._out_avals[i].shape)[c]
                for i, name in enumerate(self._out_names)
            }
            for c in range(NCORES)
        ]


_RUNNER = None


def _get_runner():
    global _RUNNER
    if _RUNNER is None:
        _RUNNER = Runner(build_program())
    return _RUNNER


def kernel(**inputs):
    runner = _get_runner()
    in_maps = shard_inputs(**inputs)
    results = runner.run(in_maps)
    return unshard(results, inputs["Wo"], inputs["bv"], inputs["bo"])


# revision 38
# speedup vs baseline: 1.0632x; 1.0632x over previous
"""Multi-head attention (B=2, S=2048, D=1024, H=16) on 8 Trainium2 cores.

Sharding: data-parallel over batch (2 groups of 4 cores) x tensor-parallel
over heads (4 heads per core, organized as 2 pairs). Each core:
  - projects its 4 heads' q/k/v (bf16 matmuls; K bias dropped -- it only
    shifts each query's scores by a constant, which softmax cancels),
  - computes scores^T = K_h Q_h^T / 8 per head with the two heads of a
    pair emitted as adjacent matmuls on PE row groups 0-1 / 2-3 (they run
    concurrently -- 2x effective rate on the 64-deep contraction),
  - exp on ScalarE in 3-bank PSUM chunks (amortizes the ~300-cycle
    per-instruction overhead),
  - attended^T = [V_h | 1]^T P^T -- the appended ones column yields the
    softmax denominators for free in PSUM row 64,
  - normalizes via one batched reciprocal per q-chunk ([4,512] covering
    all 4 heads -- DVE reciprocal is ~8 cyc/elem along the free dim, so
    batching heads 4x-es it) + rank-1 PE broadcast per pair,
  - row-parallel output projection producing a partial out^T [D, S] in
    bf16.
Host sums the 4 partials per batch, transposes, and adds the constant
bias vector bo + bv @ Wo^T (the V bias commutes through softmax because
the attention weights sum to 1).
"""

import sys

if '/opt/trn_rl_repo' not in sys.path:
    sys.path.insert(0, '/opt/trn_rl_repo')

import numpy as np

import concourse.bass as bass
import concourse.mybir as mybir
import concourse.tile as tile

# ---------------------------------------------------------------------------
# Workaround: the walrus build in this container accepts only one sync-wait
# per instruction. Hoist excess waits onto single-wait NoOp carriers, and
# emit the Tile tail-drain waits as individual SP instructions.
# ---------------------------------------------------------------------------
from concourse.vector_clock import ScopedClock

_MAXW = 1
_carrier_counter = [0]


def _split_excess_waits(tc, ordered):
    for insts in ordered.values():
        out = []
        for inst in insts:
            si = inst.sync_info
            waits = list(si.on_wait) if si is not None and si.on_wait else []
            if len(waits) > _MAXW:
                for w in waits[_MAXW:]:
                    _carrier_counter[0] += 1
                    out.append(mybir.InstNoOp(
                        name=f"I-waitcarrier-{_carrier_counter[0]}",
                        engine=inst.engine,
                        sync_info=mybir.SyncInfo(on_wait=[w], on_update=[]),
                        bass_nofuse=True,
                    ))
                inst.sync_info = mybir.SyncInfo(
                    on_wait=waits[:_MAXW],
                    on_update=list(si.on_update) if si.on_update else [],
                )
            out.append(inst)
        if len(out) != len(insts):
            insts[:] = out


class _SplitTileClockWait:
    def __init__(self, tc, ordered):
        self._w = _OrigTileClockWait(tc, ordered)
        self._tc = tc
        self._ordered = ordered

    def assign_waits(self, bb_name):
        r = self._w.assign_waits(bb_name)
        _split_excess_waits(self._tc, self._ordered)
        return r

    def __getattr__(self, name):
        return getattr(self._w, name)


def _patched_drain_and_barrier(self, tick_clock, wait_clock):
    nc = self.nc
    probe = mybir.InstNoOp(
        name=nc.get_next_instruction_name(), engine=mybir.EngineType.SP
    )
    wait_clock.add_sem_waits(probe, ScopedClock({None: tick_clock.global_clock}))
    waits = list(probe.sync_info.on_wait) if probe.sync_info is not None else []
    assert self.sems is not None
    allocated = list(self.sems.allocated().values())
    id2handle = {h.num: h for h in allocated}
    for w in waits:
        nc.sync.wait_ge(id2handle[w.id], w.wait_value)
    nc.sync.drain()
    nc.all_engine_barrier()
    popped = nc._tile_sem_poison_stack.pop()
    assert popped is self._sem_poison
    nc.clear_and_free_semaphores(allocated)
    nc.all_engine_barrier()


_OrigTileClockWait = None


def _apply_tilefix():
    global _OrigTileClockWait
    if _OrigTileClockWait is None:
        _OrigTileClockWait = tile.TileClockWait
        tile.TileClockWait = _SplitTileClockWait
        tile.TileContext._drain_and_barrier = _patched_drain_and_barrier


_apply_tilefix()

# ---------------------------------------------------------------------------
# Problem constants
# ---------------------------------------------------------------------------
F32 = mybir.dt.float32
F32R = mybir.dt.float32r
BF16 = mybir.dt.bfloat16
FP8 = mybir.dt.float8e4
EXP = mybir.ActivationFunctionType.Exp

# Q/K projections in fp8e4m3 with DoubleRow (2 contraction tiles per
# matmul). Host ships Wq/Wk scaled by 16 (their native ±1/32 range would
# land in fp8 subnormals); the 16*16 factor on the scores is folded into
# the exp() scale. V stays bf16: value-quantization error passes straight
# through to the output, score error is softened by softmax.
QK_FP8 = True
QK_SCALE = 16.0

B, S, D, H = 2, 2048, 1024, 16
DH = D // H                    # 64
NCORES = 8
GROUPS = 4                     # head groups (cores per batch)
HPG = H // GROUPS              # 4 heads per core
PAIRS = HPG // 2               # 2 head pairs per core
MW = HPG * DH                  # 256: per-core projection width
KC = D // 128                  # 8 contraction chunks for the projections
MC = MW // 128                 # 2 partition-chunks of the head dim (== pair)
QBLK = 512


def build_program(seq=S, loop_iters=None, phases=('proj', 'attn', 'out'),
                  xbufs=3, scb=3, sbufs=2, accbufs=2, ptbufs=3,
                  dup_every=0):
    """Emit the per-core Bass program. seq can be shrunk for simulation.

    scb: scores PSUM tile size in banks (matmuls per exp instruction).
    dup_every: if k>0, emit a throwaway duplicate of every k-th scores
    matmul (PE filler to keep the HAM clock-gate warm when ScalarE gates).
    """
    assert seq % QBLK == 0
    SC = seq // QBLK            # s-chunks (projection streaming)
    QC = seq // QBLK            # q-chunks (attention)
    KT = seq // 128             # key-row tiles
    ET = D // 128               # output-feature tiles

    nc = bass.Bass("TRN2", target_bir_lowering=False, debug=False,
                   num_devices=NCORES)
    dt_qk = FP8 if QK_FP8 else BF16
    xqT = nc.dram_tensor("xqT", [D, seq], dt_qk, kind="ExternalInput").ap()
    xkT = nc.dram_tensor("xkT", [D, seq], dt_qk, kind="ExternalInput").ap()
    xvT = nc.dram_tensor("xvT", [D, seq], BF16, kind="ExternalInput").ap()
    wqT = nc.dram_tensor("wqT", [D, MW], dt_qk, kind="ExternalInput").ap()
    wkT = nc.dram_tensor("wkT", [D, MW], dt_qk, kind="ExternalInput").ap()
    wvT = nc.dram_tensor("wvT", [D, MW], BF16, kind="ExternalInput").ap()
    woT = nc.dram_tensor("woT", [MW, D], BF16, kind="ExternalInput").ap()
    bq = nc.dram_tensor("bq", [MW], F32, kind="ExternalInput").ap()
    ebc = nc.dram_tensor("ebc", [128, 2 * 128], BF16,
                         kind="ExternalInput").ap()
    outT = nc.dram_tensor("outT", [D, seq], BF16, kind="ExternalOutput").ap()

    with tile.TileContext(nc) as tc:
        with (
            tc.tile_pool(name="w", bufs=1) as wpool,
            tc.tile_pool(name="x", bufs=xbufs) as xpool,
            tc.tile_pool(name="qkv", bufs=1) as qkvp,
            tc.tile_pool(name="pt", bufs=ptbufs) as ptp,
            tc.tile_pool(name="attn", bufs=2) as attnp,
            tc.tile_pool(name="io", bufs=2) as iop,
            tc.tile_pool(name="r", bufs=2) as rp,
            tc.tile_pool(name="ps", bufs=1, space="PSUM") as psp,
        ):
            def body():
                # --- weights + biases resident ---
                wq_sb = wpool.tile([128, KC, MW], dt_qk, tag="wq")
                wk_sb = wpool.tile([128, KC, MW], dt_qk, tag="wk")
                wv_sb = wpool.tile([128, KC, MW], BF16, tag="wv")
                wo_sb = wpool.tile([128, MC, D], BF16, tag="wo")
                bq_sb = wpool.tile([128, MC], F32, tag="bq")

                def load_w(kind):
                    if kind == "k":
                        nc.sync.dma_start(
                            out=wk_sb[:],
                            in_=wkT.rearrange("(kc p) m -> p kc m", p=128))
                    elif kind == "q":
                        nc.sync.dma_start(
                            out=wq_sb[:],
                            in_=wqT.rearrange("(kc p) m -> p kc m", p=128))
                        nc.sync.dma_start(
                            out=bq_sb[:],
                            in_=bq.rearrange("(mc p) -> p mc", p=128))
                    elif kind == "v":
                        nc.sync.dma_start(
                            out=wv_sb[:],
                            in_=wvT.rearrange("(kc p) m -> p kc m", p=128))
                        nc.sync.dma_start(
                            out=wo_sb[:],
                            in_=woT.rearrange("(mc p) e -> p mc e", p=128))

                qT_sb = qkvp.tile([128, MC, seq], BF16, tag="qT")
                kT_sb = qkvp.tile([128, MC, seq], BF16, tag="kT")
                v_sb = qkvp.tile([128, KT, HPG, DH + 1], BF16, tag="v")
                ones_src = wpool.tile([128, KT * HPG], F32, tag="ones")
                nc.vector.memset(ones_src[:], 1.0)
                nc.vector.tensor_copy(
                    v_sb[:, :, :, DH],
                    ones_src[:].rearrange("p (kt h) -> p kt h", h=HPG))
                # Pair-broadcast selector (host-built): Ep[p][i, c] = 1 iff
                # row i (= partition 32*h of the reciprocal tile r4) belongs
                # to the head owning partition c of pair p. rb = Ep^T @ r4
                # broadcasts each head's 1/den row across its 64 partitions.
                # Denominators sit on 32-aligned partitions because DVE
                # accesses must start on a quadrant boundary.
                e_sb = wpool.tile([128, 2, 128], BF16, tag="e")
                nc.sync.dma_start(
                    out=e_sb[:], in_=ebc.rearrange("r (mc c) -> r mc c", c=128))

                # --- projections ---
                if 'proj' not in phases:
                    return

                loaded_w = set()

                def emit_proj(kind, xdram, w_sb, scs=None):
                    if kind not in loaded_w:
                        loaded_w.add(kind)
                        load_w(kind)
                    for sc in (range(SC) if scs is None else scs):
                        x_sb = xpool.tile([128, KC, QBLK],
                                          BF16 if kind == "v" else dt_qk,
                                          tag="x" if kind == "v" else "x8")
                        nc.sync.dma_start(
                            out=x_sb[:],
                            in_=xdram.rearrange("(kc p) s -> p kc s", p=128)
                            [:, :, sc * QBLK:(sc + 1) * QBLK])
                        if 'nomm' in phases:
                            continue
                        if kind != "v":
                            dest = qT_sb if kind == "q" else kT_sb
                            w_sb = wq_sb if kind == "q" else wk_sb
                            for mc in range(MC):
                                ps = psp.tile([128, QBLK], F32, tag="acc",
                                              bufs=accbufs)
                                ms = slice(mc * 128, (mc + 1) * 128)
                                if QK_FP8:
                                    for kc2 in range(KC // 2):
                                        nc.tensor.matmul(
                                            ps[:],
                                            w_sb[:, 2 * kc2:2 * kc2 + 2, ms],
                                            x_sb[:, 2 * kc2:2 * kc2 + 2, :],
                                            start=(kc2 == 0),
                                            stop=(kc2 == KC // 2 - 1),
                                            perf_mode=(
                                                mybir.MatmulPerfMode.DoubleRow))
                                else:
                                    for kc in range(KC):
                                        nc.tensor.matmul(
                                            ps[:], w_sb[:, kc, ms],
                                            x_sb[:, kc, :],
                                            start=(kc == 0),
                                            stop=(kc == KC - 1))
                                dst = dest[:, mc, sc * QBLK:(sc + 1) * QBLK]
                                if kind == "q":
                                    nc.vector.tensor_scalar_add(
                                        dst, ps[:], bq_sb[:, mc:mc + 1])
                                else:
                                    nc.vector.tensor_copy(dst, ps[:])
                        else:
                            for st in range(QBLK // 128):
                                ps = psp.tile([128, QBLK], F32, tag="acc",
                                              bufs=accbufs)
                                for kc in range(KC):
                                    nc.tensor.matmul(
                                        ps[:, 0:MW],
                                        x_sb[:, kc, st * 128:(st + 1) * 128],
                                        wv_sb[:, kc, :],
                                        start=(kc == 0), stop=(kc == KC - 1))
                                kt = sc * (QBLK // 128) + st
                                nc.vector.tensor_copy(
                                    v_sb[:, kt, :, 0:DH],
                                    ps[:, 0:MW].rearrange(
                                        "p (h d) -> p h d", h=HPG))

                if 'attn' not in phases:
                    emit_proj("k", xkT, wk_sb)
                    emit_proj("q", xqT, wq_sb)
                    emit_proj("v", xvT, wv_sb)
                    return
                emit_proj("k", xkT, wk_sb, scs=[0])

                # --- attention ---
                def emit_scores_pair(pair, qc, pts, j_lo=0, j_hi=None):
                    """Scores + exp for both heads of a pair, interleaved so
                    the 64-contraction matmuls overlap on PE row groups.
                    j indexes (kt, head) slots; a sub-range lets the qc=0
                    scores chase the K projection chunk by chunk."""
                    if j_lo == 0:
                        pts[pair] = ptp.tile([128, 2 * KT, QBLK], BF16,
                                             tag="pt", name=f"pt{pair}_{qc}")
                    pt = pts[pair]
                    qs = slice(qc * QBLK, (qc + 1) * QBLK)
                    n = 2 * KT if j_hi is None else j_hi
                    j = j_lo
                    while j < n:
                        w = min(scb, n - j)
                        ps_s = psp.tile([128, scb, QBLK], F32, tag="s",
                                        bufs=sbufs)
                        for i in range(w):
                            kt, half = divmod(j + i, 2)
                            lo = half * 64
                            if dup_every and ((j + i) % dup_every == 0):
                                nc.tensor.matmul(
                                    ps_s[:, i, :],
                                    kT_sb[lo:lo + 64, pair,
                                          kt * 128:(kt + 1) * 128],
                                    qT_sb[lo:lo + 64, pair, qs],
                                    start=True, stop=True)
                            nc.tensor.matmul(
                                ps_s[:, i, :],
                                kT_sb[lo:lo + 64, pair,
                                      kt * 128:(kt + 1) * 128],
                                qT_sb[lo:lo + 64, pair, qs],
                                start=True, stop=True)
                        nc.scalar.activation(
                            pt[:, j:j + w, :], ps_s[:, 0:w, :],
                            EXP, scale=1.0 / np.sqrt(DH) / (
                                QK_SCALE * QK_SCALE if QK_FP8 else 1.0))
                        j += w

                def emit_pv_pair(pair, qc, pt, pv_sb, den4):
                    """P@V for both heads of a pair; values land in pv_sb
                    (head A rows 0-63, head B rows 64-127), denominators on
                    32-aligned den4 partitions 32*(2*pair) / 32*(2*pair+1)."""
                    for half in range(2):
                        h = 2 * pair + half
                        ps_pv = psp.tile([128, QBLK], F32, tag="acc",
                                         bufs=accbufs)
                        for kt in range(KT):
                            nc.tensor.matmul(
                                ps_pv[0:DH + 1, :], v_sb[:, kt, h, :],
                                pt[:, 2 * kt + half, :],
                                start=(kt == 0), stop=(kt == KT - 1))
                        nc.vector.tensor_copy(
                            pv_sb[half * 64:(half + 1) * 64, :],
                            ps_pv[0:DH, :])
                        nc.vector.tensor_copy(
                            den4[32 * h:32 * h + 1, :], ps_pv[DH:DH + 1, :])

                def emit_recip(den4, r4s, qc):
                    # One reciprocal covers all 4 heads' denominators (DVE
                    # reciprocal cost is free-dim-serial, partition-parallel;
                    # rows other than 32h hold the memset 1.0 filler). The
                    # broadcast+multiply consume it one iteration later, and
                    # it is emitted in 4 chunks: DVE reciprocal runs ~7
                    # cyc/elem and a monolithic one head-of-line-blocks the
                    # DVE queue, starving PE of freed PSUM accumulators.
                    r4 = rp.tile([128, QBLK], BF16, tag="r4", bufs=2,
                                 name=f"r4_{qc}")
                    r4s[qc] = r4
                    with nc.allow_low_precision(reason="bf16 denom bcast"):
                        for c in range(4):
                            cs = slice(c * QBLK // 4, (c + 1) * QBLK // 4)
                            nc.vector.reciprocal(r4[:, cs], den4[:, cs])

                def emit_bcmul(qc, attn_sb, pv_sbs, r4):
                    for pair in range(PAIRS):
                        rb_ps = psp.tile([128, QBLK], F32, tag="acc",
                                         bufs=accbufs)
                        nc.tensor.matmul(rb_ps[:], e_sb[:, pair, :], r4[:],
                                         start=True, stop=True)
                        nc.vector.tensor_mul(
                            attn_sb[:, pair, :], pv_sbs[pair][:], rb_ps[:])

                def emit_outproj(qc, attn_sb, ets=None, scalar_copy=False):
                    for et in (range(ET) if ets is None else ets):
                        ps_o = psp.tile([128, QBLK], F32, tag="acc",
                                        bufs=accbufs)
                        for mc in range(MC):
                            nc.tensor.matmul(
                                ps_o[:],
                                wo_sb[:, mc, et * 128:(et + 1) * 128],
                                attn_sb[:, mc, :],
                                start=(mc == 0), stop=(mc == MC - 1))
                        ot = iop.tile([128, QBLK], BF16, tag="ot")
                        if scalar_copy:
                            # tail iterations: exp stream is finished, so
                            # the idle ScalarE drains PSUM instead of DVE
                            nc.scalar.copy(ot[:], ps_o[:])
                        else:
                            nc.vector.tensor_copy(ot[:], ps_o[:])
                        nc.sync.dma_start(
                            out=outT.rearrange("(et p) q -> p et q", p=128)
                            [:, et, qc * QBLK:(qc + 1) * QBLK],
                            in_=ot[:])

                # qc=0: scores chase the K projection chunk by chunk so
                # ScalarE starts exping within the first few us; the V
                # projection and remaining Q chunks keep PE fed while
                # ScalarE works through the qc=0 exps.
                emit_proj("q", xqT, wq_sb, scs=[0])
                ptss = {0: {}}
                KJ = 2 * (QBLK // 128)   # (kt, head) slots per K s-chunk
                for sc in range(SC):
                    if sc > 0:
                        emit_proj("k", xkT, wk_sb, scs=[sc])
                    for pair in range(PAIRS):
                        emit_scores_pair(pair, 0, ptss[0],
                                         j_lo=sc * KJ, j_hi=(sc + 1) * KJ)
                emit_proj("v", xvT, wv_sb)
                if SC > 1:
                    emit_proj("q", xqT, wq_sb, scs=[1])

                attns = {}
                pvs = {}
                r4s = {}
                if loop_iters is not None:
                    tc.stage_boundary()
                for qc in range(1, QC + 1):
                    cur = qc <= QC - 1
                    if cur:
                        ptss[qc] = {}
                    prev = qc - 1
                    pvs[prev] = [
                        rp.tile([128, QBLK], F32R, tag="pv", bufs=4,
                                name=f"pv{prev}_{p}") for p in range(PAIRS)]
                    den4 = rp.tile([128, QBLK], BF16, tag="den4",
                                   bufs=2, name=f"den{prev}")
                    nc.vector.memset(den4[:], 1.0)
                    for pair in range(PAIRS):
                        if cur:
                            emit_scores_pair(pair, qc, ptss[qc])
                        emit_pv_pair(pair, prev, ptss[prev].pop(pair),
                                     pvs[prev][pair], den4)
                        if pair == 0:
                            if qc >= 2:
                                attns[qc - 2] = attnp.tile(
                                    [128, MC, QBLK], BF16, tag="attn",
                                    name=f"attn{qc - 2}")
                                emit_bcmul(qc - 2, attns[qc - 2],
                                           pvs.pop(qc - 2), r4s.pop(qc - 2))
                            if qc + 1 <= SC - 1:
                                emit_proj("q", xqT, wq_sb, scs=[qc + 1])
                            if qc >= 2 and 'out' in phases:
                                emit_outproj(qc - 2, attns[qc - 2],
                                             ets=range(ET // 2))
                        else:
                            if qc >= 2 and 'out' in phases:
                                emit_outproj(qc - 2, attns.pop(qc - 2),
                                             ets=range(ET // 2, ET),
                                             scalar_copy=(qc == QC))
                            emit_recip(den4, r4s, prev)
                    if loop_iters is not None and qc in (2, 3):
                        tc.stage_boundary()
                # tail: last q-chunk's normalize + output projection
                attns[QC - 1] = attnp.tile([128, MC, QBLK], BF16,
                                           tag="attn", name=f"attn{QC - 1}")
                emit_bcmul(QC - 1, attns[QC - 1], pvs.pop(QC - 1),
                           r4s.pop(QC - 1))
                if 'out' in phases:
                    emit_outproj(QC - 1, attns.pop(QC - 1), scalar_copy=True)

            if loop_iters is not None:
                with tc.For_i(0, loop_iters, 1,
                              staggered_reset=True,
                              hint_engines=(mybir.EngineType.PE,
                                            mybir.EngineType.DVE,
                                            mybir.EngineType.Activation,
                                            mybir.EngineType.SP)):
                    body()
            else:
                body()

    return nc


# ---------------------------------------------------------------------------
# Host-side sharding / unsharding
# ---------------------------------------------------------------------------

def shard_inputs(query, keys, values, Wq, bq, Wk, bk, Wv, bv, Wo, bo):
    import ml_dtypes
    bf16 = ml_dtypes.bfloat16
    qk_dt = ml_dtypes.float8_e4m3fn if QK_FP8 else bf16
    qk_ws = QK_SCALE if QK_FP8 else 1.0
    ebc = np.zeros((128, PAIRS, 128), np.float32)
    for pair in range(PAIRS):
        ebc[32 * (2 * pair), pair, 0:64] = 1.0
        ebc[32 * (2 * pair) + 32, pair, 64:128] = 1.0
    ebc = ebc.reshape(128, PAIRS * 128)
    in_maps = []
    for c in range(NCORES):
        b, g = divmod(c, GROUPS)
        cols = slice(g * MW, (g + 1) * MW)
        in_maps.append({
            "xqT": np.ascontiguousarray(np.asarray(query)[b].T).astype(qk_dt),
            "xkT": np.ascontiguousarray(np.asarray(keys)[b].T).astype(qk_dt),
            "xvT": np.ascontiguousarray(np.asarray(values)[b].T).astype(bf16),
            "wqT": np.ascontiguousarray(
                np.asarray(Wq)[cols].T * qk_ws).astype(qk_dt),
            "wkT": np.ascontiguousarray(
                np.asarray(Wk)[cols].T * qk_ws).astype(qk_dt),
            "wvT": np.ascontiguousarray(np.asarray(Wv)[cols].T).astype(bf16),
            "woT": np.ascontiguousarray(
                np.asarray(Wo)[:, cols].T).astype(bf16),
            "bq": np.ascontiguousarray(np.asarray(bq)[cols] * qk_ws),
            "ebc": ebc.astype(bf16),
        })
    return in_maps


def unshard(results, Wo, bv, bo):
    const = np.asarray(bo) + np.asarray(bv) @ np.asarray(Wo).T
    out = np.zeros((B, S, D), np.float32)
    for c in range(NCORES):
        b = c // GROUPS
        out[b] += results[c]["outT"].astype(np.float32).T
    out += const.astype(np.float32)
    return out


# ---------------------------------------------------------------------------
# Cached PJRT runner (compile once per process)
# ---------------------------------------------------------------------------

class Runner:
    def __init__(self, nc):
        import jax
        from concourse import bass2jax
        from jax.experimental.shard_map import shard_map
        from jax.sharding import Mesh, PartitionSpec

        bass2jax.install_neuronx_cc_hook()
        self._jax = jax
        partition_name = (nc.partition_id_tensor.name
                          if nc.partition_id_tensor else None)
        in_names, out_names, out_avals = [], [], []
        self._zero_templates = []
        for alloc in nc.m.functions[0].allocations:
            if not isinstance(alloc, mybir.MemoryLocationSet):
                continue
            name = alloc.memorylocations[0].name
            if alloc.kind == "ExternalInput":
                if name == partition_name:
                    continue
                in_names.append(name)
            elif alloc.kind == "ExternalOutput":
                out_names.append(name)
                shape = tuple(alloc.tensor_shape)
                dtype = mybir.dt.np(alloc.dtype)
                out_avals.append(jax.core.ShapedArray(shape, dtype))
                self._zero_templates.append((shape, dtype))
        self._in_names = list(in_names)
        self._out_names = list(out_names)
        self._out_avals = out_avals
        n_params = len(in_names)
        n_outs = len(out_names)
        all_in_names = in_names + out_names
        if partition_name is not None:
            all_in_names = all_in_names + [partition_name]

        def _body(*args):
            operands = list(args)
            if partition_name is not None:
                operands.append(bass2jax.partition_id_tensor())
            outs = bass2jax._bass_exec_p.bind(
                *operands,
                out_avals=tuple(out_avals),
                in_names=tuple(all_in_names),
                out_names=tuple(out_names),
                lowering_input_output_aliases=(),
                sim_require_finite=True,
                sim_require_nnan=True,
                nc=nc,
            )
            return tuple(outs)

        devices = jax.devices()[:NCORES]
        mesh = Mesh(np.asarray(devices), ("core",))
        donate = tuple(range(n_params, n_params + n_outs))
        self._fn = jax.jit(
            shard_map(_body, mesh=mesh,
                      in_specs=(PartitionSpec("core"),) * (n_params + n_outs),
                      out_specs=(PartitionSpec("core"),) * n_outs,
                      check_rep=False),
            donate_argnums=donate, keep_unused=True)

    def run(self, in_maps):
        concat_in = [
            np.concatenate([np.asarray(m[name]) for m in in_maps], axis=0)
            for name in self._in_names
        ]
        concat_zeros = [
            np.zeros((NCORES * shape[0], *shape[1:]), dtype)
            for shape, dtype in self._zero_templates
        ]
        out_arrs = self._fn(*concat_in, *concat_zeros)
        return [
            {
                name: np.asarray(out_arrs[i]).reshape(
                    NCORES, *self._out_avals[i].shape)[c]
                for i, name in enumerate(self._out_names)
            }
            for c in range(NCORES)
        ]


_RUNNER = None


def _get_runner():
    global _RUNNER
    if _RUNNER is None:
        _RUNNER = Runner(build_program())
    return _RUNNER


def kernel(**inputs):
    runner = _get_runner()
    in_maps = shard_inputs(**inputs)
    results = runner.run(in_maps)
    return unshard(results, inputs["Wo"], inputs["bv"], inputs["bo"])
